# revision 56
# baseline (speedup 1.0000x reference)
"""LocalAttention (B=4, H=16, L=2048, D=64, R=256) Trainium2 kernel.

The reference mask `(j-i >= 2048) | (j-i <= 1792)` keeps only keys with
j - i >= 1793.  Consequences (verified numerically vs the reference):
  * queries i in [0, 254] attend to the key band j in [i+1793, 2047]
    (masked logits underflow to exactly 0 after exp in f32, like the
    reference's exp(-10000 - max)),
  * queries i in [255, 2047] have every key masked -> softmax is uniform
    -> output row = mean(v over L).

So per (b, h) head we compute:
  1. mean_v = (1/2048) * sum_l v[l, :]            -> rows 255..2047
  2. a 255x255 "triangular band" attention with
     Q = q[0:255], K = k[1793:2047], V = v[1793:2047]  -> rows 0..254

Sharding: 64 (b,h) pairs, 8 per NeuronCore (data+head parallel, no
cross-device comm).  The bulk inputs are shipped bf16 (the fp32 scores
accumulate in PSUM; verified ~6e-3 relative error, well inside the
2e-2 gate), the output is produced in f32.

Performance structure (v4):
  * Every DMA uses all 128 partitions (the DMA engine spread is
    partitions/8, so 64-partition transfers run at half bandwidth):
    Q/K bands are packed two pairs per 128 partitions (d + 64*(i%2)),
    V-band/ones and the band outputs are pair-major.
  * All three DMA issue queues (SP / ACT / gpsimd-SWDGE) carry a
    balanced share; DMA transfer time occupies the issuing engine, so
    ACT (which must run exp) only issues small transfers.
  * The v mean runs as accumulating bf16 matmuls on PE with a 1/2048
    constant weight column (256-col moving dim), with a few chunks
    reduced on DVE to balance the two engines.
  * exp runs on ACT (merged two pairs per instruction through a
    2-bank PSUM score tile, bf16 output); a dummy warmup activation
    pulls the exp table load into the initial DMA fill.
  * The triangular mask is ONE precomputed [128,128] bf16 triangle
    applied by strided DVE multiplies; softmax denominators and
    normalization are merged strided DVE ops over packed PSUM banks.
  * rows 255..2047 are written by per-pair replicated-source DMAs on
    the gpsimd queue (cheap SWDGE broadcast).

NOTE this walrus build rejects instructions with more than one attached
sync wait, so `_legalize_waits` splits them into single-wait NoOps.
"""

import numpy as np
import ml_dtypes
from contextlib import ExitStack

import concourse.bass as bass
import concourse.mybir as mybir
import concourse.tile as tile
from concourse.bass_utils import run_bass_kernel_spmd

B, H, L, D = 4, 16, 2048, 64
BH = B * H            # 64 (b,h) pairs
NCORES = 8
PER = BH // NCORES    # 8 pairs per core
GRP = 4               # pairs per pipeline group
BAND = 256            # padded band (queries 0..255 / keys 1792..2047)
NQ = 255              # valid band queries (0..254)
JCH = 14              # non-band v rows per partition (1792/128)
import os
JD0 = int(os.environ.get("K_JD0", "3"))  # g0 mean j-chunks on DVE
JD1 = int(os.environ.get("K_JD1", "3"))  # g1 mean j-chunks on DVE
JSP = int(os.environ.get("K_JSP", "5"))  # v j-chunks on the SP queue

F32 = mybir.dt.float32
BF16 = mybir.dt.bfloat16
NPBF = ml_dtypes.bfloat16
EXP = mybir.ActivationFunctionType.Exp
SCALE = 0.125         # 1/sqrt(D)

VM_P = JCH * D        # 896 cols per pair in vm
QK_P = 2 * BAND       # 512 qk cols per pair
VB_P = 2 * (D + 1)    # 130 cols per pair in vbo
PB = 512              # one PSUM bank, in f32 elements


def _ap(base, extra):
    """AP with `base`'s partition dim and explicit free dims (elements)."""
    return bass.AP(tensor=base.tensor, offset=base.offset,
                   ap=[list(base.ap[0])] + extra)


def _build_bass():
    nc = bass.Bass()
    # qkT: two pairs share the 128 partitions: row 64*(i%2)+d, col
    # block i//2 holds pair i's [q0..255 | k1792..2047] transposed band
    qkT = nc.declare_dram_parameter("qkT", [128, (PER // 2) * QK_P], BF16,
                                    isOutput=False)
    vbo = nc.declare_dram_parameter("vbo", [128, PER * VB_P], BF16,
                                    isOutput=False)
    vm = nc.declare_dram_parameter("vm", [PER, 128, VM_P], BF16, isOutput=False)
    out = nc.declare_dram_parameter("out", [PER, L, D], F32, isOutput=True)

    def dram_ap(t, offset, dims):
        return bass.AP(tensor=t[0:1, 0:1, 0:1].tensor if t is vm
                       else t[0:1, 0:1].tensor, offset=offset, ap=dims)

    with tile.TileContext(nc) as tc:
        with ExitStack() as ctx:
            vpool = ctx.enter_context(tc.tile_pool(name="vpool", bufs=2))
            qkp = ctx.enter_context(tc.tile_pool(name="qkp", bufs=2))
            vbp = ctx.enter_context(tc.tile_pool(name="vbp", bufs=2))
            ep = ctx.enter_context(tc.tile_pool(name="ep", bufs=3))
            sre = ctx.enter_context(tc.tile_pool(name="sre", bufs=2))
            obp = ctx.enter_context(tc.tile_pool(name="obp", bufs=2))
            small = ctx.enter_context(tc.tile_pool(name="small", bufs=4))
            const = ctx.enter_context(tc.tile_pool(name="const", bufs=1))
            ps_pair = ctx.enter_context(tc.tile_pool(name="ps_pair", bufs=3, space="PSUM"))
            ps_m = ctx.enter_context(tc.tile_pool(name="ps_m", bufs=2, space="PSUM"))

            # ---- constants / warmup (once) ----
            w1 = const.tile([128, 1], BF16)         # 1/L weight column
            nc.vector.memset(w1, 1.0 / float(L))
            ones = const.tile([128, 128], BF16)
            nc.vector.memset(ones, 1.0)
            mask = const.tile([128, 128], BF16)     # keep iff p >= f+1
            nc.gpsimd.affine_select(
                out=mask, in_=ones, compare_op=mybir.AluOpType.is_ge,
                fill=0.0, base=-1, channel_multiplier=1, pattern=[[-1, 128]])

            # packed Q|K bands, both groups upfront: g0 on ACT, g1 on SP
            # so ACT can run the table-load warmup exp right after g0's
            # band arrives (ahead of the first scores)
            qkgs = []
            for g, eng in ((0, nc.scalar), (1, nc.gpsimd)):
                qkg = qkp.tile([128, 2 * QK_P], BF16, tag="qkg")
                eng.dma_start(
                    out=qkg,
                    in_=dram_ap(qkT, g * 2 * QK_P,
                                [[(PER // 2) * QK_P, 128], [1, 2 * QK_P]]))
                qkgs.append(qkg)
            # dummy exp pulls the ACT table load into the DMA fill phase
            warm = const.tile([1, 1], F32)
            nc.scalar.activation(warm, ones[0:1, 0:1], EXP)

            for g in range(2):
                p0 = g * GRP
                qkg = qkgs[g]
                # ---------------- group loads ----------------
                # v rows 0:1792 of 4 pairs, j-split between SP and gpsimd
                vg = vpool.tile([128, GRP * VM_P], BF16, tag="vg")
                vg4 = vg.rearrange("p (i j d) -> p i j d", i=GRP, j=JCH)
                for j0, j1, eng in ((0, JSP, nc.sync), (JSP, JCH, nc.gpsimd)):
                    src = dram_ap(vm, p0 * 128 * VM_P + j0 * D,
                                  [[VM_P, 128], [128 * VM_P, GRP],
                                   [D, j1 - j0], [1, D]])
                    eng.dma_start(out=vg4[:, :, j0:j1, :], in_=src)
                # V band + ones of 4 pairs, pair-major (SP queue)
                vbg = vbp.tile([128, GRP * VB_P], BF16, tag="vbg")
                nc.sync.dma_start(
                    out=vbg,
                    in_=dram_ap(vbo, g * GRP * VB_P,
                                [[PER * VB_P, 128], [1, GRP * VB_P]]))
                vbg3 = vbg.rearrange("p (i c) -> p i c", i=GRP)

                # ---------------- band attention ----------------
                # One PSUM bank per pair, exactly packed: scores for
                # (k0, q0..126) at cols 0:127, (k1, q0..254) at 127:382
                # (q127's k0 block and q255 are fully masked and never
                # computed), u0 (q0..126) at 382:447, u1 (q127..254) at
                # 447:512 -- each with the ones-column denominator.
                for i2 in range(2):
                    pp = ps_pair.tile([128, 2 * PB], F32, tag="pp")
                    e = ep.tile([128, 2 * 382], BF16, tag="e")
                    for ii in range(2):
                        i = 2 * i2 + ii
                        qk = qkg[64 * (i % 2):64 * (i % 2) + 64,
                                 (i // 2) * QK_P:(i // 2 + 1) * QK_P]
                        c0 = ii * PB
                        nc.tensor.matmul(pp[:, c0:c0 + 127],
                                         lhsT=qk[:, BAND:BAND + 128],
                                         rhs=qk[:, 0:127],
                                         start=True, stop=True)
                        nc.tensor.matmul(pp[:, c0 + 127:c0 + 382],
                                         lhsT=qk[:, BAND + 128:2 * BAND],
                                         rhs=qk[:, 0:NQ],
                                         start=True, stop=True)
                    # exp(score/8) -> bf16 for two pairs in one ACT op
                    nc.scalar.activation(
                        e.rearrange("p (ii c) -> p ii c", ii=2),
                        _ap(pp[0:128, 0], [[PB, 2], [1, 382]]),
                        EXP, scale=SCALE)
                    # zero all four triangles (cols 0:127 / 255:382 of both
                    # pairs) with one strided bf16 multiply on DVE
                    ev2 = _ap(e[0:128, 0], [[382, 2], [255, 2], [1, 127]])
                    nc.vector.tensor_mul(
                        out=ev2, in0=ev2,
                        in1=_ap(mask[0:128, 0], [[0, 2], [0, 2], [1, 127]]))
                    for ii in range(2):
                        i = 2 * i2 + ii
                        # U = P^T V with the softmax denominator in col D
                        e0 = ii * 382
                        c0 = ii * PB
                        u0 = pp[:, c0 + 382:c0 + 382 + D + 1]
                        u1 = pp[:, c0 + 447:c0 + 447 + D + 1]
                        # k1 block first (128 cols incl q127) so u0's
                        # partition 127 is initialized; q127 itself is
                        # stored from the u1 half
                        nc.tensor.matmul(u0, lhsT=e[:, e0 + 127:e0 + 255],
                                         rhs=vbg3[:, i, D + 1:VB_P],
                                         start=True, stop=False)
                        # col e0+127 is (k1, q0) data: partition 127 of u0
                        # accumulates garbage, but stays finite and is only
                        # a pad row (q127 is stored from the u1 half)
                        nc.tensor.matmul(u0, lhsT=e[:, e0:e0 + 128],
                                         rhs=vbg3[:, i, 0:D + 1],
                                         start=False, stop=True)
                        nc.tensor.matmul(u1, lhsT=e[:, e0 + 254:e0 + 382],
                                         rhs=vbg3[:, i, D + 1:VB_P],
                                         start=True, stop=True)
                    # normalize both pairs on DVE (merged strided views;
                    # every computed query has >=1 live key, so den > 0)
                    r = small.tile([128, 4], F32, tag="r")
                    ob = obp.tile([128, 2 * 2 * D], F32, tag="ob")
                    obv = ob.rearrange("p (i h d) -> p i h d", i=2, h=2)
                    nc.vector.reciprocal(
                        r.rearrange("p (i h) -> p i h", i=2),
                        _ap(pp[0:128, 382 + D], [[PB, 2], [65, 2], [1, 1]]))
                    nc.vector.tensor_mul(
                        out=obv,
                        in0=_ap(pp[0:128, 382], [[PB, 2], [65, 2], [1, D]]),
                        in1=_ap(r[0:128, 0], [[2, 2], [1, 2], [0, D]]))
                    # band stores: rows 0..126 from u0-half, 127..254 from
                    # u1-half (SP + ACT tails)
                    ob4 = ob.rearrange("p (i h d) -> p i h d", i=2, h=2)
                    o0 = (p0 + 2 * i2) * L * D
                    ST_ENG = {"s": nc.sync, "a": nc.scalar, "g": nc.gpsimd}
                    st_plan = os.environ.get("K_ST", "ssga")
                    ST_ENG[st_plan[2 * i2]].dma_start(
                        out=bass.AP(tensor=out[0:1, 0:1, 0:1].tensor,
                                    offset=o0,
                                    ap=[[D, 127], [L * D, 2], [1, D]]),
                        in_=ob4[0:127, :, 0, :])
                    ST_ENG[st_plan[2 * i2 + 1]].dma_start(
                        out=bass.AP(tensor=out[0:1, 0:1, 0:1].tensor,
                                    offset=o0 + 127 * D,
                                    ap=[[D, 128], [L * D, 2], [1, D]]),
                        in_=ob4[:, :, 1, :])

                # ---------------- mean(v) ----------------
                # (low priority: the mean only feeds the Pool bcast queue,
                # while PE must favor the scores -> exp -> AV spine)
                # (1/L)*ones^T @ v: JD chunks pre-reduced on DVE, the rest
                # accumulated on PE (256-col bf16 rhs -> 1 cyc/row)
                JD = JD0 if g == 0 else JD1
                lp = tc.high_priority(offset=-(10 ** 6))
                lp.__enter__()
                if JD:
                    red = sre.tile([128, GRP * D], BF16, tag="red")
                    with nc.allow_low_precision(
                            reason="JD-chunk partial sums; ~1e-4 of mean"):
                        for i in range(GRP):
                            rv = vg4[0:128, i, 0, 0]
                            nc.vector.reduce_sum(
                                out=red.rearrange(
                                    "p (i d) -> p i d",
                                    i=GRP)[:, i:i + 1, :, None],
                                in_=_ap(rv, [[1, 1], [1, D], [D, JD]]),
                                axis=mybir.AxisListType.X)
                mean_ps = ps_m.tile([1, GRP * D], F32, tag="m")
                lhs = w1[:, :]
                for n, c in enumerate(range(JD, JCH)):
                    nc.tensor.matmul(mean_ps, lhsT=lhs, rhs=vg4[:, :, c, :],
                                     start=(n == 0), stop=False)
                for h in range(2):
                    rhs = _ap(vbg3[:, 0, h * (D + 1)], [[VB_P, GRP], [1, D]])
                    nc.tensor.matmul(mean_ps, lhsT=lhs, rhs=rhs,
                                     start=False, stop=(h == 1 and not JD))
                if JD:
                    nc.tensor.matmul(mean_ps, lhsT=lhs, rhs=red,
                                     start=False, stop=True)
                mean_sb = small.tile([1, GRP * D], F32, tag="msb")
                if g == 0:
                    nc.scalar.copy(mean_sb, mean_ps)
                else:
                    nc.vector.tensor_copy(mean_sb, mean_ps)
                # broadcast mean rows to out rows 255..2047 (replicated
                # source, one cheap SWDGE DMA per pair)
                BC_ENG = {"s": nc.sync, "a": nc.scalar, "g": nc.gpsimd}
                bc_plan = os.environ.get("K_BC", "ggsa")
                for i, eng in enumerate(BC_ENG[c] for c in bc_plan):
                    msb = mean_sb[0:1, i * D:(i + 1) * D]
                    eng.dma_start(
                        out=out[p0 + i, NQ:L, :],
                        in_=_ap(msb, [[0, L - NQ], [1, D]]))
                lp.__exit__(None, None, None)

    return nc


def _legalize_waits(nc):
    """This walrus build rejects instructions carrying more than one
    attached sync wait (per-struct slot limits, e.g. PE Matmult and the
    kernel-tail Drain).  Split every multi-wait instruction's waits into
    preceding single-wait NoOps on the same engine queue — same-queue
    ordering preserves semantics exactly."""
    n = 0
    for fn in nc.m.functions:
        for blk in fn.blocks:
            new_insts = []
            for inst in blk.instructions:
                si = inst.sync_info
                if si is not None and si.on_wait and len(si.on_wait) > 1:
                    for w in si.on_wait:
                        n += 1
                        new_insts.append(mybir.InstNoOp(
                            name=f"legwait-{n}",
                            engine=inst.engine,
                            ins=[], outs=[],
                            sync_info=mybir.SyncInfo(on_wait=[w], on_update=[]),
                            bass_nofuse=True,
                        ))
                    inst.sync_info = mybir.SyncInfo(
                        on_wait=[], on_update=list(si.on_update or []))
                new_insts.append(inst)
            blk.instructions[:] = new_insts


_NC = None
_LEGALIZED = False


def _get_nc(legalize=False):
    global _NC, _LEGALIZED
    if _NC is None:
        _NC = _build_bass()
    if legalize and not _LEGALIZED:
        # CoreSim chokes on the injected NoOps, so only legalize for the
        # HW compile path
        _legalize_waits(_NC)
        _LEGALIZED = True
    return _NC


def _make_in_maps(q, k, v):
    qf = np.asarray(q, dtype=np.float32).reshape(BH, L, D)
    kf = np.asarray(k, dtype=np.float32).reshape(BH, L, D)
    vf = np.asarray(v, dtype=np.float32).reshape(BH, L, D)
    # host-side marshalling: transpose the Q/K bands and pack two pairs
    # per 128 partitions; V band packed pair-major with ones-columns;
    # vm is a reshape view.  Bulk tensors ship as bf16.
    qkT = np.concatenate(
        [qf[:, 0:BAND, :].transpose(0, 2, 1),
         kf[:, L - BAND:L, :].transpose(0, 2, 1)], axis=2)  # [BH, 64, 512]
    qkT = (qkT.reshape(NCORES, PER // 2, 2, D, QK_P)
           .transpose(0, 2, 3, 1, 4)
           .reshape(NCORES, 128, (PER // 2) * QK_P)).astype(NPBF)
    vband = vf[:, L - BAND:L, :].reshape(BH, 2, 128, D)  # [BH, 2, 128, 64]
    vbo = np.ones((BH, 128, VB_P), dtype=np.float32)
    vbo[:, :, 0:D] = vband[:, 0]
    vbo[:, :, D + 1:2 * D + 1] = vband[:, 1]
    vbo = (vbo.reshape(NCORES, PER, 128, VB_P)
           .transpose(0, 2, 1, 3)
           .reshape(NCORES, 128, PER * VB_P)).astype(NPBF)
    vm = vf[:, 0:128 * JCH, :].reshape(BH, 128, VM_P).astype(NPBF)
    in_maps = []
    for c in range(NCORES):
        in_maps.append({
            "qkT": np.ascontiguousarray(qkT[c]),
            "vbo": np.ascontiguousarray(vbo[c]),
            "vm": np.ascontiguousarray(vm[c * PER:(c + 1) * PER]),
        })
    return in_maps


def _run(q, k, v, **kwargs):
    nc = _get_nc(legalize=True)
    in_maps = _make_in_maps(q, k, v)
    return run_bass_kernel_spmd(nc, in_maps, list(range(NCORES)), **kwargs)


def kernel(q, k, v):
    res = _run(q, k, v)
    outs = [res.results[c]["out"] for c in range(NCORES)]
    return np.concatenate(outs, axis=0).reshape(B, H, L, D)


# revision 57
# speedup vs baseline: 1.0112x; 1.0112x over previous
"""LocalAttention (B=4, H=16, L=2048, D=64, R=256) Trainium2 kernel.

The reference mask `(j-i >= 2048) | (j-i <= 1792)` keeps only keys with
j - i >= 1793.  Consequences (verified numerically vs the reference):
  * queries i in [0, 254] attend to the key band j in [i+1793, 2047]
    (masked logits underflow to exactly 0 after exp in f32, like the
    reference's exp(-10000 - max)),
  * queries i in [255, 2047] have every key masked -> softmax is uniform
    -> output row = mean(v over L).

So per (b, h) head we compute:
  1. mean_v = (1/2048) * sum_l v[l, :]            -> rows 255..2047
  2. a 255x255 "triangular band" attention with
     Q = q[0:255], K = k[1793:2047], V = v[1793:2047]  -> rows 0..254

Sharding: 64 (b,h) pairs, 8 per NeuronCore (data+head parallel, no
cross-device comm).  The bulk inputs are shipped bf16 (the fp32 scores
accumulate in PSUM; verified ~6e-3 relative error, well inside the
2e-2 gate), the output is produced in f32.

Performance structure (v4):
  * Every DMA uses all 128 partitions (the DMA engine spread is
    partitions/8, so 64-partition transfers run at half bandwidth):
    Q/K bands are packed two pairs per 128 partitions (d + 64*(i%2)),
    V-band/ones and the band outputs are pair-major.
  * All three DMA issue queues (SP / ACT / gpsimd-SWDGE) carry a
    balanced share; DMA transfer time occupies the issuing engine, so
    ACT (which must run exp) only issues small transfers.
  * The v mean runs as accumulating bf16 matmuls on PE with a 1/2048
    constant weight column (256-col moving dim), with a few chunks
    reduced on DVE to balance the two engines.
  * exp runs on ACT (merged two pairs per instruction through a
    2-bank PSUM score tile, bf16 output); a dummy warmup activation
    pulls the exp table load into the initial DMA fill.
  * The triangular mask is ONE precomputed [128,128] bf16 triangle
    applied by strided DVE multiplies; softmax denominators and
    normalization are merged strided DVE ops over packed PSUM banks.
  * rows 255..2047 are written by per-pair replicated-source DMAs on
    the gpsimd queue (cheap SWDGE broadcast).

NOTE this walrus build rejects instructions with more than one attached
sync wait, so `_legalize_waits` splits them into single-wait NoOps.
"""

import numpy as np
import ml_dtypes
from contextlib import ExitStack

import concourse.bass as bass
import concourse.mybir as mybir
import concourse.tile as tile
from concourse.bass_utils import run_bass_kernel_spmd

B, H, L, D = 4, 16, 2048, 64
BH = B * H            # 64 (b,h) pairs
NCORES = 8
PER = BH // NCORES    # 8 pairs per core
GRP = 4               # pairs per pipeline group
BAND = 256            # padded band (queries 0..255 / keys 1792..2047)
NQ = 255              # valid band queries (0..254)
JCH = 14              # non-band v rows per partition (1792/128)
import os
JD0 = int(os.environ.get("K_JD0", "3"))  # g0 mean j-chunks on DVE
JD1 = int(os.environ.get("K_JD1", "3"))  # g1 mean j-chunks on DVE
JSP = int(os.environ.get("K_JSP", "5"))  # v j-chunks on the SP queue

F32 = mybir.dt.float32
BF16 = mybir.dt.bfloat16
NPBF = ml_dtypes.bfloat16
EXP = mybir.ActivationFunctionType.Exp
SCALE = 0.125         # 1/sqrt(D)

VM_P = JCH * D        # 896 cols per pair in vm
QK_P = 2 * BAND       # 512 qk cols per pair
VB_P = 2 * (D + 1)    # 130 cols per pair in vbo
PB = 512              # one PSUM bank, in f32 elements


def _ap(base, extra):
    """AP with `base`'s partition dim and explicit free dims (elements)."""
    return bass.AP(tensor=base.tensor, offset=base.offset,
                   ap=[list(base.ap[0])] + extra)


def _build_bass():
    nc = bass.Bass()
    # qkT: two pairs share the 128 partitions: row 64*(i%2)+d, col
    # block i//2 holds pair i's [q0..255 | k1792..2047] transposed band
    qkT = nc.declare_dram_parameter("qkT", [128, (PER // 2) * QK_P], BF16,
                                    isOutput=False)
    vbo = nc.declare_dram_parameter("vbo", [128, PER * VB_P], BF16,
                                    isOutput=False)
    vm = nc.declare_dram_parameter("vm", [PER, 128, VM_P], BF16, isOutput=False)
    out = nc.declare_dram_parameter("out", [PER, L, D], F32, isOutput=True)

    def dram_ap(t, offset, dims):
        return bass.AP(tensor=t[0:1, 0:1, 0:1].tensor if t is vm
                       else t[0:1, 0:1].tensor, offset=offset, ap=dims)

    with tile.TileContext(nc) as tc:
        with ExitStack() as ctx:
            vpool = ctx.enter_context(tc.tile_pool(name="vpool", bufs=2))
            qkp = ctx.enter_context(tc.tile_pool(name="qkp", bufs=2))
            vbp = ctx.enter_context(tc.tile_pool(name="vbp", bufs=2))
            ep = ctx.enter_context(tc.tile_pool(name="ep", bufs=3))
            sre = ctx.enter_context(tc.tile_pool(name="sre", bufs=2))
            obp = ctx.enter_context(tc.tile_pool(name="obp", bufs=2))
            small = ctx.enter_context(tc.tile_pool(name="small", bufs=4))
            const = ctx.enter_context(tc.tile_pool(name="const", bufs=1))
            ps_pair = ctx.enter_context(tc.tile_pool(name="ps_pair", bufs=3, space="PSUM"))
            ps_m = ctx.enter_context(tc.tile_pool(name="ps_m", bufs=2, space="PSUM"))

            # ---- constants / warmup (once) ----
            w1 = const.tile([128, 1], BF16)         # 1/L weight column
            nc.vector.memset(w1, 1.0 / float(L))
            ones = const.tile([128, 128], BF16)
            nc.vector.memset(ones, 1.0)
            mask = const.tile([128, 128], BF16)     # keep iff p >= f+1
            nc.gpsimd.affine_select(
                out=mask, in_=ones, compare_op=mybir.AluOpType.is_ge,
                fill=0.0, base=-1, channel_multiplier=1, pattern=[[-1, 128]])

            # packed Q|K bands, both groups upfront: g0 on ACT, g1 on SP
            # so ACT can run the table-load warmup exp right after g0's
            # band arrives (ahead of the first scores)
            qkgs = []
            for g, eng in ((0, nc.scalar), (1, nc.gpsimd)):
                qkg = qkp.tile([128, 2 * QK_P], BF16, tag="qkg")
                eng.dma_start(
                    out=qkg,
                    in_=dram_ap(qkT, g * 2 * QK_P,
                                [[(PER // 2) * QK_P, 128], [1, 2 * QK_P]]))
                qkgs.append(qkg)
            # dummy exp pulls the ACT table load into the DMA fill phase
            warm = const.tile([1, 1], F32)
            nc.scalar.activation(warm, ones[0:1, 0:1], EXP)

            for g in range(2):
                p0 = g * GRP
                qkg = qkgs[g]
                # ---------------- group loads ----------------
                # v rows 0:1792 of 4 pairs, j-split between SP and gpsimd
                vg = vpool.tile([128, GRP * VM_P], BF16, tag="vg")
                vg4 = vg.rearrange("p (i j d) -> p i j d", i=GRP, j=JCH)
                for j0, j1, eng in ((0, JSP, nc.sync), (JSP, JCH, nc.gpsimd)):
                    src = dram_ap(vm, p0 * 128 * VM_P + j0 * D,
                                  [[VM_P, 128], [128 * VM_P, GRP],
                                   [D, j1 - j0], [1, D]])
                    eng.dma_start(out=vg4[:, :, j0:j1, :], in_=src)
                # V band + ones of 4 pairs, pair-major (SP queue)
                vbg = vbp.tile([128, GRP * VB_P], BF16, tag="vbg")
                nc.sync.dma_start(
                    out=vbg,
                    in_=dram_ap(vbo, g * GRP * VB_P,
                                [[PER * VB_P, 128], [1, GRP * VB_P]]))
                vbg3 = vbg.rearrange("p (i c) -> p i c", i=GRP)

                # ---------------- band attention ----------------
                # One PSUM bank per pair, exactly packed: scores for
                # (k0, q0..126) at cols 0:127, (k1, q0..254) at 127:382
                # (q127's k0 block and q255 are fully masked and never
                # computed), u0 (q0..126) at 382:447, u1 (q127..254) at
                # 447:512 -- each with the ones-column denominator.
                for i2 in range(2):
                    pp = ps_pair.tile([128, 2 * PB], F32, tag="pp")
                    e = ep.tile([128, 2 * 382], BF16, tag="e")
                    for ii in range(2):
                        i = 2 * i2 + ii
                        qk = qkg[64 * (i % 2):64 * (i % 2) + 64,
                                 (i // 2) * QK_P:(i // 2 + 1) * QK_P]
                        c0 = ii * PB
                        nc.tensor.matmul(pp[:, c0:c0 + 127],
                                         lhsT=qk[:, BAND:BAND + 128],
                                         rhs=qk[:, 0:127],
                                         start=True, stop=True)
                        nc.tensor.matmul(pp[:, c0 + 127:c0 + 382],
                                         lhsT=qk[:, BAND + 128:2 * BAND],
                                         rhs=qk[:, 0:NQ],
                                         start=True, stop=True)
                    # exp(score/8) -> bf16 for two pairs in one ACT op
                    nc.scalar.activation(
                        e.rearrange("p (ii c) -> p ii c", ii=2),
                        _ap(pp[0:128, 0], [[PB, 2], [1, 382]]),
                        EXP, scale=SCALE)
                    # zero all four triangles (cols 0:127 / 255:382 of both
                    # pairs) with one strided bf16 multiply; the two middle
                    # blocks run on the idle gpsimd engine so DVE's window
                    # stays clear for the buffer-freeing normalizes
                    ev2 = _ap(e[0:128, 0], [[382, 2], [255, 2], [1, 127]])
                    mk2 = _ap(mask[0:128, 0], [[0, 2], [0, 2], [1, 127]])
                    meng = nc.gpsimd if (g, i2) in ((0, 1), (1, 0))                         else nc.vector
                    meng.tensor_mul(out=ev2, in0=ev2, in1=mk2)
                    for ii in range(2):
                        i = 2 * i2 + ii
                        # U = P^T V with the softmax denominator in col D
                        e0 = ii * 382
                        c0 = ii * PB
                        u0 = pp[:, c0 + 382:c0 + 382 + D + 1]
                        u1 = pp[:, c0 + 447:c0 + 447 + D + 1]
                        # k1 block first (128 cols incl q127) so u0's
                        # partition 127 is initialized; q127 itself is
                        # stored from the u1 half
                        nc.tensor.matmul(u0, lhsT=e[:, e0 + 127:e0 + 255],
                                         rhs=vbg3[:, i, D + 1:VB_P],
                                         start=True, stop=False)
                        # col e0+127 is (k1, q0) data: partition 127 of u0
                        # accumulates garbage, but stays finite and is only
                        # a pad row (q127 is stored from the u1 half)
                        nc.tensor.matmul(u0, lhsT=e[:, e0:e0 + 128],
                                         rhs=vbg3[:, i, 0:D + 1],
                                         start=False, stop=True)
                        nc.tensor.matmul(u1, lhsT=e[:, e0 + 254:e0 + 382],
                                         rhs=vbg3[:, i, D + 1:VB_P],
                                         start=True, stop=True)
                    # normalize both pairs on DVE (merged strided views;
                    # every computed query has >=1 live key, so den > 0)
                    r = small.tile([128, 4], F32, tag="r")
                    ob = obp.tile([128, 2 * 2 * D], F32, tag="ob")
                    obv = ob.rearrange("p (i h d) -> p i h d", i=2, h=2)
                    nc.vector.reciprocal(
                        r.rearrange("p (i h) -> p i h", i=2),
                        _ap(pp[0:128, 382 + D], [[PB, 2], [65, 2], [1, 1]]))
                    nc.vector.tensor_mul(
                        out=obv,
                        in0=_ap(pp[0:128, 382], [[PB, 2], [65, 2], [1, D]]),
                        in1=_ap(r[0:128, 0], [[2, 2], [1, 2], [0, D]]))
                    # band stores: rows 0..126 from u0-half, 127..254 from
                    # u1-half (SP + ACT tails)
                    ob4 = ob.rearrange("p (i h d) -> p i h d", i=2, h=2)
                    o0 = (p0 + 2 * i2) * L * D
                    ST_ENG = {"s": nc.sync, "a": nc.scalar, "g": nc.gpsimd}
                    st_plan = os.environ.get("K_ST", "ssga")
                    ST_ENG[st_plan[2 * i2]].dma_start(
                        out=bass.AP(tensor=out[0:1, 0:1, 0:1].tensor,
                                    offset=o0,
                                    ap=[[D, 127], [L * D, 2], [1, D]]),
                        in_=ob4[0:127, :, 0, :])
                    ST_ENG[st_plan[2 * i2 + 1]].dma_start(
                        out=bass.AP(tensor=out[0:1, 0:1, 0:1].tensor,
                                    offset=o0 + 127 * D,
                                    ap=[[D, 128], [L * D, 2], [1, D]]),
                        in_=ob4[:, :, 1, :])

                # ---------------- mean(v) ----------------
                # (low priority: the mean only feeds the Pool bcast queue,
                # while PE must favor the scores -> exp -> AV spine)
                # (1/L)*ones^T @ v: JD chunks pre-reduced on DVE, the rest
                # accumulated on PE (256-col bf16 rhs -> 1 cyc/row)
                JD = JD0 if g == 0 else JD1
                lp = tc.high_priority(offset=-(10 ** 6))
                lp.__enter__()
                if JD:
                    red = sre.tile([128, GRP * D], BF16, tag="red")
                    with nc.allow_low_precision(
                            reason="JD-chunk partial sums; ~1e-4 of mean"):
                        for i in range(GRP):
                            rv = vg4[0:128, i, 0, 0]
                            nc.vector.reduce_sum(
                                out=red.rearrange(
                                    "p (i d) -> p i d",
                                    i=GRP)[:, i:i + 1, :, None],
                                in_=_ap(rv, [[1, 1], [1, D], [D, JD]]),
                                axis=mybir.AxisListType.X)
                mean_ps = ps_m.tile([1, GRP * D], F32, tag="m")
                lhs = w1[:, :]
                for n, c in enumerate(range(JD, JCH)):
                    nc.tensor.matmul(mean_ps, lhsT=lhs, rhs=vg4[:, :, c, :],
                                     start=(n == 0), stop=False)
                for h in range(2):
                    rhs = _ap(vbg3[:, 0, h * (D + 1)], [[VB_P, GRP], [1, D]])
                    nc.tensor.matmul(mean_ps, lhsT=lhs, rhs=rhs,
                                     start=False, stop=(h == 1 and not JD))
                if JD:
                    nc.tensor.matmul(mean_ps, lhsT=lhs, rhs=red,
                                     start=False, stop=True)
                mean_sb = small.tile([1, GRP * D], F32, tag="msb")
                if g == 0:
                    nc.scalar.copy(mean_sb, mean_ps)
                else:
                    nc.vector.tensor_copy(mean_sb, mean_ps)
                # broadcast mean rows to out rows 255..2047 (replicated
                # source, one cheap SWDGE DMA per pair)
                BC_ENG = {"s": nc.sync, "a": nc.scalar, "g": nc.gpsimd}
                bc_plan = os.environ.get("K_BC", "ggsa")
                for i, eng in enumerate(BC_ENG[c] for c in bc_plan):
                    msb = mean_sb[0:1, i * D:(i + 1) * D]
                    eng.dma_start(
                        out=out[p0 + i, NQ:L, :],
                        in_=_ap(msb, [[0, L - NQ], [1, D]]))
                lp.__exit__(None, None, None)

    return nc


def _legalize_waits(nc):
    """This walrus build rejects instructions carrying more than one
    attached sync wait (per-struct slot limits, e.g. PE Matmult and the
    kernel-tail Drain).  Split every multi-wait instruction's waits into
    preceding single-wait NoOps on the same engine queue — same-queue
    ordering preserves semantics exactly."""
    n = 0
    for fn in nc.m.functions:
        for blk in fn.blocks:
            new_insts = []
            for inst in blk.instructions:
                si = inst.sync_info
                if si is not None and si.on_wait and len(si.on_wait) > 1:
                    for w in si.on_wait:
                        n += 1
                        new_insts.append(mybir.InstNoOp(
                            name=f"legwait-{n}",
                            engine=inst.engine,
                            ins=[], outs=[],
                            sync_info=mybir.SyncInfo(on_wait=[w], on_update=[]),
                            bass_nofuse=True,
                        ))
                    inst.sync_info = mybir.SyncInfo(
                        on_wait=[], on_update=list(si.on_update or []))
                new_insts.append(inst)
            blk.instructions[:] = new_insts


_NC = None
_LEGALIZED = False


def _get_nc(legalize=False):
    global _NC, _LEGALIZED
    if _NC is None:
        _NC = _build_bass()
    if legalize and not _LEGALIZED:
        # CoreSim chokes on the injected NoOps, so only legalize for the
        # HW compile path
        _legalize_waits(_NC)
        _LEGALIZED = True
    return _NC


def _make_in_maps(q, k, v):
    qf = np.asarray(q, dtype=np.float32).reshape(BH, L, D)
    kf = np.asarray(k, dtype=np.float32).reshape(BH, L, D)
    vf = np.asarray(v, dtype=np.float32).reshape(BH, L, D)
    # host-side marshalling: transpose the Q/K bands and pack two pairs
    # per 128 partitions; V band packed pair-major with ones-columns;
    # vm is a reshape view.  Bulk tensors ship as bf16.
    qkT = np.concatenate(
        [qf[:, 0:BAND, :].transpose(0, 2, 1),
         kf[:, L - BAND:L, :].transpose(0, 2, 1)], axis=2)  # [BH, 64, 512]
    qkT = (qkT.reshape(NCORES, PER // 2, 2, D, QK_P)
           .transpose(0, 2, 3, 1, 4)
           .reshape(NCORES, 128, (PER // 2) * QK_P)).astype(NPBF)
    vband = vf[:, L - BAND:L, :].reshape(BH, 2, 128, D)  # [BH, 2, 128, 64]
    vbo = np.ones((BH, 128, VB_P), dtype=np.float32)
    vbo[:, :, 0:D] = vband[:, 0]
    vbo[:, :, D + 1:2 * D + 1] = vband[:, 1]
    vbo = (vbo.reshape(NCORES, PER, 128, VB_P)
           .transpose(0, 2, 1, 3)
           .reshape(NCORES, 128, PER * VB_P)).astype(NPBF)
    vm = vf[:, 0:128 * JCH, :].reshape(BH, 128, VM_P).astype(NPBF)
    in_maps = []
    for c in range(NCORES):
        in_maps.append({
            "qkT": np.ascontiguousarray(qkT[c]),
            "vbo": np.ascontiguousarray(vbo[c]),
            "vm": np.ascontiguousarray(vm[c * PER:(c + 1) * PER]),
        })
    return in_maps


def _run(q, k, v, **kwargs):
    nc = _get_nc(legalize=True)
    in_maps = _make_in_maps(q, k, v)
    return run_bass_kernel_spmd(nc, in_maps, list(range(NCORES)), **kwargs)


def kernel(q, k, v):
    res = _run(q, k, v)
    outs = [res.results[c]["out"] for c in range(NCORES)]
    return np.concatenate(outs, axis=0).reshape(B, H, L, D)
